# revision 23
# baseline (speedup 1.0000x reference)
"""Cross-attention kernel for 8 Trainium2 NeuronCores (Bass/Tile).

Sharding: (L, head-half) tensor parallel. Core c handles batch l = c//2 and
heads [4*(c%2), 4*(c%2)+4) for ALL 2048 queries. Each core projects Q/K/V
only for its 4 heads and emits a PARTIAL output y_part = attn_out @
Wo[:, head-block]^T; the host sums the two partials of each pair and adds
the bias during unshard.

v2 changes over the original baseline (224us):
  * key compaction: the host permutes the sequence so valid keys come
    first; only M = ceil(maxvalid/128)*128 keys (typ. 1920 of 2048) enter
    K/V/scores/exp/AV. Queries keep all 2048 rows; the host un-permutes
    the output. Saves ~6% of both PE and ACT work.
  * the key-padding mask moved out of the exp bias into the V matrix:
    vp rows (including the ones/denominator column) of masked keys are
    zeroed via a tensor_mul in the V-projection PSUM->SBUF copy, so
    exp needs no per-key bias and masked keys contribute exactly 0 to
    both numerator and denominator.
  * reciprocal_approx_fast (~0.6us) replaces the 2.9us iterative DVE
    reciprocal that head-of-line-blocked the vector queue (the cause of
    mid-sweep PE stalls -> HAM re-throttles to 1.2GHz).
  * the two per-block broadcast matmuls merge into one (0/1 selector).
  * input DMA split per (b, d-chunk) and reordered; minimal presweep;
    junk matmuls paced through the head to keep the PE clock at 2.4GHz.

Per-core dataflow (matmuls in bf16 with f32 PSUM accumulation):
  qT[e, n], kT[e, m]       transposed projections (features on partitions)
  v'[mk, h, 65]            v projection * mask + a mask column per head
  sT[mk, nq] = kT.T @ qT   per head -- keys on partitions; head pairs run
                           row-packed (concurrent) on the PE
  attnT = exp(SCALE*sT)    one ACT op per chunk, no bias
  out'[65, nq] = v'.T @ attnT   rows 0..63: head out^T, row 64: denominator
  normalize: rrb = approx-recip(denoms); bc = sel.T @ rrb broadcasts both
    heads' 1/denom over 128 partitions in one matmul; DVE muls stage ->
    out_allT
  y_part = out_allT.T @ WoT     final projection (no bias; host adds it)
"""

import numpy as np
import ml_dtypes
from contextlib import ExitStack

import concourse.bass as bass
import concourse.tile as tile
from concourse import bacc, mybir
from concourse.bass_utils import run_bass_kernel_spmd

L, N, D_IN = 4, 2048, 1024
H, DH = 8, 64
INNER = H * DH          # 512
D_OUT = D_IN
SCALE = DH ** -0.5      # 0.125
NCORES = 8
HH = H // 2             # 4 heads per core
HI = HH * DH            # 256 inner features per core
DC = D_IN // 128        # 8 contraction chunks for the projections
EC = HI // 128          # 2 feature chunks (= head pairs) per core
NB = N // 512           # 4 query 512-blocks
NQB = NB                # 4 query blocks per core (all 2048 queries)

BF = mybir.dt.bfloat16
F32 = mybir.dt.float32
EXP = mybir.ActivationFunctionType.Exp


def _emit(ctx, tc, kcm, xT, wqT, wkT, wvT, woT, maskv, out):
    nc = tc.nc
    M = kcm * 128                  # compacted key count
    KB = (M + 511) // 512          # key 512-blocks (last may be partial)

    const = ctx.enter_context(tc.tile_pool(name="const", bufs=1))
    big = ctx.enter_context(tc.tile_pool(name="big", bufs=1))
    attn_sb = ctx.enter_context(tc.tile_pool(name="attn_sb", bufs=4))
    norm_sb = ctx.enter_context(tc.tile_pool(name="norm_sb", bufs=2))
    stage_sb = ctx.enter_context(tc.tile_pool(name="stage_sb", bufs=4))
    out_sb = ctx.enter_context(tc.tile_pool(name="out_sb", bufs=4))
    ps_st = ctx.enter_context(tc.tile_pool(name="ps_st", bufs=2, space="PSUM"))
    ps_o = ctx.enter_context(tc.tile_pool(name="ps_o", bufs=2, space="PSUM"))
    ps_f = ctx.enter_context(tc.tile_pool(name="ps_f", bufs=2, space="PSUM"))

    # ---- inputs -> SBUF. Order matters: the presweep needs wk + x(b0)
    # first, then wv+maskv (V presweep), then wq (Q presweep); the x tail
    # blocks land while block 0 runs. x is split per (b, d) so projections
    # can start as chunks arrive.
    wk_s = const.tile([128, DC, HI], BF)
    wq_s = const.tile([128, DC, HI], BF)
    wv_s = const.tile([128, DC, HI], BF)
    wo_s = const.tile([128, EC, D_OUT], BF)
    maskv_s = const.tile([128, kcm, HI], BF)
    xT_s = big.tile([128, NB, DC, 512], BF)
    vp_s = big.tile([128, kcm, HH, DH + 1], BF)

    nc.sync.dma_start(wk_s, wkT)
    nc.sync.dma_start(xT_s[:, 0, 0:4], xT[0, :, 0:4])
    nc.sync.dma_start(xT_s[:, 0, 4:8], xT[0, :, 4:8])
    nc.sync.dma_start(wq_s, wqT)
    nc.sync.dma_start(wv_s, wvT)
    nc.sync.dma_start(maskv_s, maskv)
    for b in range(1, NB):
        nc.sync.dma_start(xT_s[:, b], xT[b])
    nc.sync.dma_start(wo_s, woT)

    kT_s = big.tile([128, EC, M], BF)
    qT_s = big.tile([128, EC, N], BF)
    out_allT = big.tile([128, EC, N], BF)
    # persistent reciprocal input; rows other than 0 and 32 stay 1.0
    # forever (32-partition alignment is required for sliced DVE ops)
    r2 = big.tile([33, 512], F32)
    nc.vector.memset(r2, 1.0)
    # 0/1 selector: bc = sel.T @ rrb broadcasts rrb row0 over out partitions
    # 0:64 and row32 over 64:128 in a single FD=512 matmul
    sel = const.tile([33, 128], BF)
    nc.vector.memset(sel, 0.0)
    nc.vector.memset(sel[0:1, 0:64], 1.0)
    nc.vector.memset(sel[32:33, 64:128], 1.0)

    _half_open = {}

    def proj_kT(j, b, half=None):
        # half=0/1 splits the unit across two fill slots so a single slot
        # never absorbs a whole 8-matmul projection (which would starve
        # the ACT engine for ~1.3us). NOTE with ps_f bufs=2 at most one
        # other ps_f alloc may happen between the two halves.
        w = min(512, M - b * 512)
        if half == 1:
            ps = _half_open.pop(("k", j, b))
        else:
            ps = ps_f.tile([128, 512], F32, tag="f", name=f"ps_k{j}{b}")
        ds = range(DC) if half is None else (
            range(DC // 2) if half == 0 else range(DC // 2, DC))
        for d in ds:
            nc.tensor.matmul(
                ps[:, 0:w], wk_s[:, d, j * 128:(j + 1) * 128],
                xT_s[:, b, d, 0:w],
                start=(d == 0), stop=(d == DC - 1))
        if half == 0:
            _half_open[("k", j, b)] = ps
        else:
            nc.vector.tensor_copy(kT_s[:, j, b * 512:b * 512 + w], ps[:, 0:w])

    def proj_qT(j, b, half=None):
        if half == 1:
            ps = _half_open.pop(("q", j, b))
        else:
            ps = ps_f.tile([128, 512], F32, tag="f", name=f"ps_q{j}{b}")
        ds = range(DC) if half is None else (
            range(DC // 2) if half == 0 else range(DC // 2, DC))
        for d in ds:
            nc.tensor.matmul(
                ps, wq_s[:, d, j * 128:(j + 1) * 128],
                xT_s[:, b, d, :],
                start=(d == 0), stop=(d == DC - 1))
        if half == 0:
            _half_open[("q", j, b)] = ps
        else:
            nc.vector.tensor_copy(qT_s[:, j, b * 512:(b + 1) * 512], ps)

    def proj_v(c):
        # v' = (x @ Wv^T) * mask -- masked keys (incl. compaction padding)
        # get all-zero rows so they vanish from numerator AND denominator;
        # the per-head ones/denominator column gets the raw mask value
        ps = ps_f.tile([128, 512], F32, tag="f", name=f"ps_v{c}")
        for d in range(DC):
            nc.tensor.matmul(
                ps[:, 0:HI],
                xT_s[:, c // 4, d, (c % 4) * 128:(c % 4) * 128 + 128],
                wv_s[:, d, :],
                start=(d == 0), stop=(d == DC - 1))
        nc.vector.tensor_mul(
            vp_s[:, c, :, 0:DH],
            ps[:, 0:HI].rearrange("p (h e) -> p h e", h=HH),
            maskv_s[:, c, :].rearrange("p (h e) -> p h e", h=HH))
        nc.vector.tensor_copy(
            vp_s[:, c, :, DH],
            maskv_s[:, c, :].rearrange("p (h e) -> p h e", h=HH)[:, :, 0])

    # ---- warmup: junk matmuls lift the PE p-state clock gate during the
    # input DMA; a junk exp pulls the ACT table load off the critical path.
    # wps borrows a score-PSUM slot (idle until the sweep starts).
    warm = const.tile([128, 512], BF)
    nc.vector.memset(warm, 1.0)
    wps = ps_st.tile([128, 512], F32, tag="st", name="wps")
    warm_out = const.tile([1, 32], BF)

    def junk(n):
        for i in range(n):
            nc.tensor.matmul(wps, warm[:, 0:128], warm, start=(i == 0),
                             stop=(i == n - 1))

    # presweep (DMA-gated): just enough K/Q for block 0 to start scoring;
    # junk keeps the PE HAM window busy across the x(b0) DMA wait.
    junk(6)
    nc.scalar.activation(warm_out, wps[0:1, 0:32], EXP, bias=0.0, scale=0.0)
    proj_kT(0, 0, half=0)
    junk(3)
    proj_kT(0, 0, half=1)
    proj_qT(0, 0, half=0)
    proj_qT(0, 0, half=1)

    def norm_recip():
        # 1/denominator for both heads of the pending block; approx-fast
        # (~18 good bits, result is cast to bf16 anyway) keeps the DVE
        # queue short so it never gates the PE
        rr = norm_sb.tile([33, 512], F32, tag="rr", name="rr")
        nc.vector.reciprocal_approx_fast(rr, r2)
        rrb = norm_sb.tile([33, 512], BF, tag="rrb", name="rrb")
        nc.vector.tensor_copy(rrb, rr)
        return rrb

    def norm_apply(p, qb, sA, sB, rrb):
        # out_allT[head rows] = staged out' * (1/denominator); one matmul
        # broadcasts both heads' 1/denom over the 128 partitions
        bc = ps_f.tile([128, 512], F32, tag="f", name="bc")
        nc.tensor.matmul(bc, sel, rrb, start=True, stop=True)
        nc.vector.tensor_mul(
            out_allT[0:64, p, qb * 512:(qb + 1) * 512], sA, bc[0:64, :])
        nc.vector.tensor_mul(
            out_allT[64:128, p, qb * 512:(qb + 1) * 512], sB, bc[64:128, :])

    def outproj_t(j, t, tail=False):
        # out-proj contribution of head pair j alone (summed on the host)
        of = out_sb.tile([128, D_OUT], BF, tag="of", name="of")
        for f in range(D_OUT // 512):
            # tail po's borrow the score-PSUM banks (idle once the sweep is
            # done) so four projections can be in flight instead of two
            pool = ps_st if tail and f == 1 else ps_f
            tg = "st" if tail and f == 1 else "f"
            po = pool.tile([128, 512], F32, tag=tg, name=f"po{j}{t}{f}")
            nc.tensor.matmul(
                po, out_allT[:, j, t * 128:(t + 1) * 128],
                wo_s[:, j, f * 512:(f + 1) * 512], start=True, stop=True)
            # in the tail the idle Scalar engine takes half the casts so the
            # DVE is not the serial bottleneck of the last tiles
            if tail and f == 1:
                nc.scalar.copy(of[:, f * 512:(f + 1) * 512], po)
            else:
                nc.vector.tensor_copy(of[:, f * 512:(f + 1) * 512], po)
        nc.sync.dma_start(out[j][t * 128:(t + 1) * 128, :], of)

    def K(j, b):
        return lambda: proj_kT(j, b)

    def Q(j, b):
        return lambda: proj_qT(j, b)

    def O(j, t):
        return lambda: outproj_t(j, t)

    # per-block fill plan: "early" units pop at c=3,5,7,9, "late" at
    # c=kcm,kcm+1. Deadlines: K(0,b) before block-0 chunk 4b; Q(0,qb)
    # before block qb; K(1,*)/Q(1,*) before blocks 4..7; O(j,t) one block
    # after head pair j's normalize for t's query block popped (norm for
    # block bi pops at block bi+1 c=11, so O is legal from bi+1's late
    # slots onward).
    def Kh(j, b, h):
        return lambda: proj_kT(j, b, half=h)

    def Qh(j, b, h):
        return lambda: proj_qT(j, b, half=h)

    # early entries are (slot_offset, unit); late entries fire at
    # kcm-1, kcm, ... K/Q units in blocks 1+ are split into half-units on
    # adjacent slots so one slot never carries a whole projection.
    FILLS = {
        0: ([(3, K(0, 1)), (5, K(0, 2)), (7, K(0, 3)), (9, Q(0, 1))], []),
        1: ([(3, Qh(0, 2, 0)), (4, Qh(0, 2, 1)), (5, Kh(1, 0, 0)),
             (6, Kh(1, 0, 1)), (7, Kh(1, 1, 0)), (8, Kh(1, 1, 1)),
             (9, Qh(1, 0, 0)), (10, Qh(1, 0, 1))], [O(0, 0), O(0, 1)]),
        2: ([(3, Qh(0, 3, 0)), (4, Qh(0, 3, 1)), (5, Kh(1, 2, 0)),
             (6, Kh(1, 2, 1)), (7, Kh(1, 3, 0)), (8, Kh(1, 3, 1))],
            [O(0, 2), O(0, 3)]),
        3: ([(3, Qh(1, 1, 0)), (4, Qh(1, 1, 1)), (6, O(0, 4)),
             (8, O(0, 5))], [O(0, 6), O(0, 7)]),
        4: ([(3, Qh(1, 2, 0)), (4, Qh(1, 2, 1)), (6, O(0, 8)),
             (8, O(0, 9))], [O(0, 10), O(0, 11)]),
        5: ([(3, Qh(1, 3, 0)), (4, Qh(1, 3, 1)), (6, O(0, 12)),
             (8, O(0, 13))], [O(0, 14), O(0, 15)]),
        6: ([(3, O(1, 0)), (5, O(1, 1)), (7, O(1, 2)), (9, O(1, 3))],
            [O(1, 4), O(1, 5)]),
        7: ([(3, O(1, 6)), (5, O(1, 7))],
            [O(1, 8), O(1, 9), O(1, 10), O(1, 11)]),
    }

    # attention sweep, flattened into one global chunk stream: chunk g's
    # score pair + exp issue at slot g while chunk g-2's AV matmuls (which
    # may belong to the previous block) issue in the same slot. Fusing the
    # block transitions this way keeps scores flowing to the ACT engine
    # through block boundaries instead of idling it behind the AV tail.
    pending = []        # (p, qb, sA, sB, rrb) through the two norm stages
    at_l = [None] * 4
    o_acc = {}
    NBLK = NQB * EC
    TOT = NBLK * kcm
    fill_sched = {}
    for bi, (early, late) in FILLS.items():
        for off, u in early:
            fill_sched.setdefault(bi * kcm + off, []).append(u)
        for k, u in enumerate(late):
            slot = min(bi * kcm + kcm - 1 + k, TOT + 1)
            fill_sched.setdefault(slot, []).append(u)
    for g in range(TOT + 2):
        # scores + exp first in each slot: the exp for chunk g enters the
        # ACT queue before this slot's AV/fill matmuls, so the ACT engine
        # is never gated on PE work that could have waited
        if g < TOT:
            bi, c = g // kcm, g % kcm
            p, qb = bi // NQB, bi % NQB
            sT = ps_st.tile([128, 1024], F32, tag="st", name="sT")
            nc.tensor.matmul(
                sT[:, 0:512],
                kT_s[0:64, p, c * 128:(c + 1) * 128],
                qT_s[0:64, p, qb * 512:(qb + 1) * 512],
                start=True, stop=True)
            nc.tensor.matmul(
                sT[:, 512:1024],
                kT_s[64:128, p, c * 128:(c + 1) * 128],
                qT_s[64:128, p, qb * 512:(qb + 1) * 512],
                start=True, stop=True)
            at = attn_sb.tile([128, 1024], BF, tag="at", name="at")
            at_l[g % 4] = at
            nc.scalar.activation(at, sT, EXP, bias=0.0, scale=SCALE)
        if g >= 2:
            ga = g - 2
            bi, c = ga // kcm, ga % kcm
            p = bi // NQB
            hA, hB = 2 * p, 2 * p + 1
            if c == 0:
                o_acc[bi] = (
                    ps_o.tile([DH + 1, 512], F32, tag="o", name=f"oA{bi}"),
                    ps_o.tile([DH + 1, 512], F32, tag="o", name=f"oB{bi}"))
            oA, oB = o_acc[bi]
            nc.tensor.matmul(oA, vp_s[:, c, hA, :],
                             at_l[ga % 4][:, 0:512],
                             start=(c == 0), stop=(c == kcm - 1))
            nc.tensor.matmul(oB, vp_s[:, c, hB, :],
                             at_l[ga % 4][:, 512:1024],
                             start=(c == 0), stop=(c == kcm - 1))
            if c == kcm - 1:
                # stage the accumulators to SBUF (frees the PSUM slots) and
                # grab the denominator rows for the upcoming reciprocal
                del o_acc[bi]
                sA = stage_sb.tile([DH, 512], F32, tag="sA", name="sA")
                sB = stage_sb.tile([DH, 512], F32, tag="sB", name="sB")
                if bi == NBLK - 1:
                    nc.scalar.copy(r2[0:1, :], oA[DH:DH + 1, :])
                    nc.scalar.copy(r2[32:33, :], oB[DH:DH + 1, :])
                nc.vector.tensor_copy(sA, oA[0:DH, :])
                nc.vector.tensor_copy(sB, oB[0:DH, :])
                if bi != NBLK - 1:
                    nc.vector.tensor_copy(r2[0:1, :], oA[DH:DH + 1, :])
                    nc.vector.tensor_copy(r2[32:33, :], oB[DH:DH + 1, :])
                pending.append((p, bi % NQB, sA, sB, None))
        if g < TOT:
            bi, c = g // kcm, g % kcm
            if bi == 0:
                proj_v(c)
            if c == 1 and pending and pending[0][4] is None:
                pending[0] = pending[0][:4] + (norm_recip(),)
            if c == 11 and pending:
                pp, pqb, sA, sB, rrb = pending.pop(0)
                norm_apply(pp, pqb, sA, sB, rrb)
        for u in fill_sched.pop(g, []):
            u()

    # ---- tail: fast reciprocal for the last block; two output tiles that
    # only depend on the PREVIOUS block's normalize fill the PE meanwhile.
    pp, pqb, sA, sB, _ = pending.pop(0)
    # tail warm-keeper: borrow a score-PSUM slot (free once the last exp
    # has read it) so the junk does not wait on an outproj cast like a
    # ps_f slot would
    jps = ps_st.tile([128, 512], F32, tag="st", name="jps")
    for i in range(14):
        nc.tensor.matmul(jps, warm[:, 0:128], warm, start=(i == 0),
                         stop=(i == 13))
    rrb = norm_recip()
    norm_apply(pp, pqb, sA, sB, rrb)
    for t in range(12, 16):
        outproj_t(1, t, tail=True)


def _build(kcm):
    nc = bacc.Bacc("TRN2", target_bir_lowering=False, debug=False,
                   num_devices=NCORES)
    aps = dict(
        xT=nc.dram_tensor("xT", [NB, 128, DC, 512], BF,
                          kind="ExternalInput").ap(),
        wqT=nc.dram_tensor("wqT", [128, DC, HI], BF, kind="ExternalInput").ap(),
        wkT=nc.dram_tensor("wkT", [128, DC, HI], BF, kind="ExternalInput").ap(),
        wvT=nc.dram_tensor("wvT", [128, DC, HI], BF, kind="ExternalInput").ap(),
        woT=nc.dram_tensor("woT", [128, EC, D_OUT], BF,
                           kind="ExternalInput").ap(),
        maskv=nc.dram_tensor("maskv", [128, kcm, HI], BF,
                             kind="ExternalInput").ap(),
        out=nc.dram_tensor("out", [EC, N, D_OUT], BF,
                           kind="ExternalOutput").ap(),
    )
    with tile.TileContext(nc) as tc:
        with ExitStack() as ctx:
            _emit(ctx, tc, kcm, **aps)
    nc.compile()
    return nc


_progs = {}


def _get_prog(kcm):
    if kcm not in _progs:
        _progs[kcm] = _build(kcm)
    return _progs[kcm]


def _plan(mask):
    """Per-l key-compaction permutation: valid keys first. Returns kcm,
    perms, and inverse perms. kcm covers the max valid count over l."""
    mask = np.asarray(mask)
    perms, invs = [], []
    maxvalid = 0
    for l in range(L):
        m = mask[l].astype(bool)
        perm = np.argsort(~m, kind="stable").astype(np.int64)
        inv = np.empty(N, np.int64)
        inv[perm] = np.arange(N)
        perms.append(perm)
        invs.append(inv)
        maxvalid = max(maxvalid, int(m.sum()))
    kcm = max(1, min(N // 128, -(-maxvalid // 128)))
    # the fill schedule needs the early slots at c=3..9 to exist
    if kcm < 12:
        kcm = 12
    return kcm, perms, invs


def _make_in_maps(x, Wq, Wk, Wv, Wo, bo, mask, kcm=None, perms=None):
    bf = ml_dtypes.bfloat16
    if kcm is None:
        kcm, perms, _ = _plan(mask)
    M = kcm * 128

    def wlayout(w):
        # [256 out, in] -> partition-major [128, in//128, 256]
        t = np.asarray(w).T.astype(bf).reshape(-1, 128, w.shape[0])
        return np.ascontiguousarray(t.transpose(1, 0, 2))

    in_maps = []
    for c in range(NCORES):
        l, hh = c // 2, c % 2
        sl = slice(hh * HI, (hh + 1) * HI)
        xp = np.asarray(x[l])[perms[l]]
        xTl = np.ascontiguousarray(
            xp.T.astype(bf).reshape(DC, 128, NB, 512).transpose(2, 1, 0, 3))
        mp = np.zeros(M, np.float32)
        nvalid = min(M, N)
        mp[:nvalid] = np.asarray(mask[l])[perms[l]][:nvalid].astype(np.float32)
        mkey = mp.reshape(kcm, 128).T.astype(bf)          # [128, kcm]
        maskv = np.ascontiguousarray(
            np.broadcast_to(mkey[:, :, None], (128, kcm, HI)))
        woT = np.ascontiguousarray(
            Wo[:, sl].T.astype(bf).reshape(EC, 128, D_OUT).transpose(1, 0, 2))
        in_maps.append(dict(xT=xTl, wqT=wlayout(Wq[sl]), wkT=wlayout(Wk[sl]),
                            wvT=wlayout(Wv[sl]), woT=woT, maskv=maskv))
    return in_maps


def run(x, Wq, Wk, Wv, Wo, bo, mask, trace=False, tmpdir=None):
    kcm, perms, invs = _plan(mask)
    nc = _get_prog(kcm)
    in_maps = _make_in_maps(x, Wq, Wk, Wv, Wo, bo, mask, kcm=kcm, perms=perms)
    res = run_bass_kernel_spmd(nc, in_maps, core_ids=list(range(NCORES)),
                               trace=trace, tmpdir=tmpdir)
    out = np.empty((L, N, D_OUT), np.float32)
    bo_f = np.asarray(bo, np.float32)
    for l in range(L):
        a = res.results[2 * l]["out"].astype(np.float32)
        b = res.results[2 * l + 1]["out"].astype(np.float32)
        full = a[0] + a[1] + b[0] + b[1] + bo_f
        out[l] = full[invs[l]]
    return out, res


def kernel(x, Wq, Wk, Wv, Wo, bo, mask):
    out, _ = run(np.asarray(x, np.float32), np.asarray(Wq, np.float32),
                 np.asarray(Wk, np.float32), np.asarray(Wv, np.float32),
                 np.asarray(Wo, np.float32), np.asarray(bo, np.float32),
                 np.asarray(mask))
    return out
